# Initial kernel scaffold
#
"""EFLSTM Trainium2 kernel: 8-core tensor-parallel LSTM + fused head.

Strategy (8 NeuronCores, SPMD single program, per-core data differs):
  - Tensor-parallel over the 4H gate dimension: core k owns gate rows
    [k*128:(k+1)*128] of each of the four gates (i, f, g, o), so it computes
    h rows [k*128:(k+1)*128] each timestep.  A per-step AllGather rebuilds
    the full transposed hidden state h_T [H=1024, B=128] on every core.
  - Input projection x @ W_ih.T is folded into the recurrent PSUM
    accumulation (x is pre-transposed on the host with a ones-row so the
    bias is folded into the matmul as well).  The projection matmuls do not
    depend on h, so they execute during the AllGather latency window.
  - Matmul operands and the h exchange are bf16 (fp32 PSUM accumulation,
    fp32 cell state); gate nonlinearities are split per gate chunk so the
    c-update overlaps the o-gate matmuls.
  - FC head is fused per-step: fc1 (one 128-wide slice of C per core; cores
    k and k+4 duplicate a slice, fc2 weights pre-scaled by 0.5 so the final
    ReduceScatter sum is correct) + fc2 partials accumulate into a
    token-major buffer; after the loop one ReduceScatter + log_softmax on
    each core's token shard produces the output.
"""

import numpy as np
import ml_dtypes

import concourse.bacc as bacc
import concourse.mybir as mybir
import concourse.tile as tile
from concourse.bass_utils import run_bass_kernel_spmd

F32 = mybir.dt.float32
BF16 = mybir.dt.bfloat16
AF = mybir.ActivationFunctionType
ALU = mybir.AluOpType

N_CORES = 8
B = 128
T = 512
DIMS = (300, 74, 35)
D = sum(DIMS)  # 409
DP = D + 1     # 410: extra ones-row folds the gate bias into the matmul
H = 1024
G = 4 * H
C = 512
O = 7
HSL = H // N_CORES        # 128 h rows per core
GSL = 4 * HSL             # 512 gate rows per core
KC_X = [128, 128, 128, DP - 3 * 128]   # contraction chunks over D+1
NKX = len(KC_X)
NKH = H // 128            # 8 contraction chunks over H
TOK = B * T


NO_CC = False


def build_kernel(t_steps=T):
    nc = bacc.Bacc("TRN2", target_bir_lowering=False, debug=False,
                   num_devices=N_CORES)

    xT = nc.dram_tensor("xT", [t_steps, NKX, 128, B], BF16, kind="ExternalInput")
    wihT = nc.dram_tensor("wihT", [NKX, 128, GSL], BF16, kind="ExternalInput")
    whhT = nc.dram_tensor("whhT", [NKH, 128, GSL], BF16, kind="ExternalInput")
    fc1wT = nc.dram_tensor("fc1wT", [NKH, 128, 128], BF16, kind="ExternalInput")
    fc1b = nc.dram_tensor("fc1b", [128, 1], F32, kind="ExternalInput")
    fc2wT = nc.dram_tensor("fc2wT", [128, O], BF16, kind="ExternalInput")
    fc2b = nc.dram_tensor("fc2b", [128, O], F32, kind="ExternalInput")

    n_tok_loc = B * t_steps // N_CORES
    out_sh = nc.dram_tensor("out_sh", [n_tok_loc, O], F32, kind="ExternalOutput")

    with tile.TileContext(nc) as tc:
        with (
            tc.tile_pool(name="const", bufs=1) as const,
            tc.tile_pool(name="xtp", bufs=3) as xtp,
            tc.tile_pool(name="pg", bufs=2, space="PSUM") as pgp,
            tc.tile_pool(name="ph", bufs=2, space="PSUM") as php,
            tc.tile_pool(name="pf", bufs=2, space="PSUM") as pfp,
            tc.tile_pool(name="ew", bufs=2) as ewp,
            tc.tile_pool(name="htp", bufs=3) as htp,
            tc.tile_pool(name="dram", bufs=2, space="DRAM") as dramp,
            tc.tile_pool(name="dram1", bufs=1, space="DRAM") as dramp1,
        ):
            # ---- resident weights ----
            wih_sb = const.tile([128, NKX, GSL], BF16)
            nc.sync.dma_start(wih_sb[:], wihT[:].rearrange("k p g -> p k g"))
            whh_sb = const.tile([128, NKH, GSL], BF16)
            nc.sync.dma_start(whh_sb[:], whhT[:].rearrange("k p g -> p k g"))
            fc1_sb = const.tile([128, NKH, 128], BF16)
            nc.sync.dma_start(fc1_sb[:], fc1wT[:].rearrange("k p c -> p k c"))
            fc1b_sb = const.tile([128, 1], F32)
            nc.sync.dma_start(fc1b_sb[:], fc1b[:])
            fc2w_sb = const.tile([128, O], BF16)
            nc.sync.dma_start(fc2w_sb[:], fc2wT[:])
            fc2b_sb = const.tile([128, O], F32)
            nc.sync.dma_start(fc2b_sb[:], fc2b[:])

            c_sb = const.tile([128, 128], F32)   # persistent cell state slice

            out_part = dramp1.tile([n_tok_loc * N_CORES, O], F32)

            hT_prev = None
            gate_slices = [(mc * 128, (mc + 1) * 128) for mc in range(4)]

            for t in range(t_steps):
                # -- projection matmuls for step t (independent of h) --
                xts = xtp.tile([128, NKX, B], BF16)
                nc.sync.dma_start(xts[:], xT[t].rearrange("k p b -> p k b"))
                pg = pgp.tile([128, GSL], F32)
                for mc in range(4):
                    lo, hi = gate_slices[mc]
                    for kc in range(NKX):
                        kk = KC_X[kc]
                        nc.tensor.matmul(
                            pg[:, lo:hi],
                            wih_sb[:kk, kc, lo:hi],
                            xts[:kk, kc, :],
                            start=(kc == 0),
                            stop=(t == 0 and kc == NKX - 1),
                        )
                # -- recurrent matmuls (consume previous gathered h) --
                if t > 0:
                    for mc in range(4):
                        lo, hi = gate_slices[mc]
                        for kc in range(NKH):
                            nc.tensor.matmul(
                                pg[:, lo:hi],
                                whh_sb[:, kc, lo:hi],
                                hT_prev[:, kc, :],
                                start=False,
                                stop=(kc == NKH - 1),
                            )

                # -- head for step t-1 (emitted after rec so PE prioritizes rec) --
                if t > 0:
                    emit_head(nc, php, pfp, ewp, fc1_sb, fc1b_sb, fc2w_sb,
                              hT_prev, out_part, t - 1, t_steps)

                # -- elementwise: gates -> h slice (i,f,g,o gate chunk order;
                #    c-update runs while the o-gate matmuls finish) --
                gnl = ewp.tile([128, GSL], F32)
                nc.scalar.activation(gnl[:, 0:256], pg[:, 0:256], AF.Sigmoid)
                if t > 0:
                    fcs = ewp.tile([128, 128], F32)
                    nc.vector.tensor_mul(fcs[:], gnl[:, 128:256], c_sb[:])
                nc.scalar.activation(gnl[:, 256:384], pg[:, 256:384], AF.Tanh)
                ig = ewp.tile([128, 128], F32)
                nc.vector.tensor_mul(ig[:], gnl[:, 0:128], gnl[:, 256:384])
                if t == 0:
                    nc.vector.tensor_copy(c_sb[:], ig[:])
                else:
                    nc.vector.tensor_add(c_sb[:], fcs[:], ig[:])
                tc_t = ewp.tile([128, 128], F32)
                nc.scalar.activation(tc_t[:], c_sb[:], AF.Tanh)
                nc.scalar.activation(gnl[:, 384:512], pg[:, 384:512], AF.Sigmoid)
                h_sl = ewp.tile([128, 128], BF16)
                nc.vector.tensor_mul(h_sl[:], gnl[:, 384:512], tc_t[:])

                # -- exchange: AllGather h_T across the 8 cores --
                bnc_in = dramp.tile([128, B], BF16)
                nc.sync.dma_start(bnc_in[:], h_sl[:])
                bnc_out = dramp.tile(
                    [H, B], BF16, addr_space="Local" if NO_CC else "Shared")
                if NO_CC:
                    for kc in range(NKH):
                        nc.gpsimd.dma_start(
                            bnc_out[kc * 128:(kc + 1) * 128, :], bnc_in[:])
                else:
                    nc.gpsimd.collective_compute(
                        "AllGather",
                        ALU.bypass,
                        replica_groups=[list(range(N_CORES))],
                        ins=[bnc_in[:].opt()],
                        outs=[bnc_out[:].opt()],
                    )
                hT = htp.tile([128, NKH, B], BF16)
                for kc in range(NKH):
                    nc.sync.dma_start(hT[:, kc, :],
                                      bnc_out[kc * 128:(kc + 1) * 128, :])
                hT_prev = hT

            # head for the final step
            emit_head(nc, php, pfp, ewp, fc1_sb, fc1b_sb, fc2w_sb,
                      hT_prev, out_part, t_steps - 1, t_steps)

            # ---- tail: ReduceScatter fc2 partials, bias + log_softmax ----
            rs_out = dramp1.tile([n_tok_loc, O], F32)
            nc.gpsimd.collective_compute(
                "ReduceScatter",
                ALU.add,
                replica_groups=[list(range(N_CORES))],
                ins=[out_part[:].opt()],
                outs=[rs_out[:].opt()],
            )
            n_chunks = n_tok_loc // 128
            for ch in range(n_chunks):
                z = ewp.tile([128, O], F32)
                nc.sync.dma_start(z[:], rs_out[ch * 128:(ch + 1) * 128, :])
                zb = ewp.tile([128, O], F32)
                nc.vector.tensor_add(zb[:], z[:], fc2b_sb[:])
                mx = ewp.tile([128, 1], F32)
                nc.vector.reduce_max(mx[:], zb[:], axis=mybir.AxisListType.X)
                sh = ewp.tile([128, O], F32)
                nc.vector.tensor_scalar_sub(sh[:], zb[:], mx[:])
                ex = ewp.tile([128, O], F32)
                nc.scalar.activation(ex[:], sh[:], AF.Exp)
                sm = ewp.tile([128, 1], F32)
                nc.vector.reduce_sum(sm[:], ex[:], axis=mybir.AxisListType.X)
                lg = ewp.tile([128, 1], F32)
                nc.scalar.activation(lg[:], sm[:], AF.Ln)
                res = ewp.tile([128, O], F32)
                nc.vector.tensor_scalar_sub(res[:], sh[:], lg[:])
                nc.sync.dma_start(out_sh[ch * 128:(ch + 1) * 128, :], res[:])

    nc.compile()
    return nc


def emit_head(nc, php, pfp, ewp, fc1_sb, fc1b_sb, fc2w_sb, hT, out_part, t,
              t_steps):
    """fc1 (C slice) + relu + fc2 partial for timestep t, token-major store."""
    ph = php.tile([128, B], F32)
    for kc in range(NKH):
        nc.tensor.matmul(ph[:], fc1_sb[:, kc, :], hT[:, kc, :],
                         start=(kc == 0), stop=(kc == NKH - 1))
    hid = ewp.tile([128, B], BF16)
    nc.scalar.activation(hid[:], ph[:], AF.Relu, bias=fc1b_sb[:])
    pf = pfp.tile([128, O], F32)
    nc.tensor.matmul(pf[:], hid[:], fc2w_sb[:], start=True, stop=True)
    z = ewp.tile([128, O], F32)
    nc.vector.tensor_copy(z[:], pf[:])
    # out_part is token-major [(b t), O]
    nc.sync.dma_start(
        out_part[:].rearrange("(b t) o -> b t o", t=t_steps)[:, t, :], z[:])


_CACHED = {}


def _get_kernel(t_steps):
    if t_steps not in _CACHED:
        _CACHED[t_steps] = build_kernel(t_steps)
    return _CACHED[t_steps]


def prep_inputs(m_text, m_audio, m_video, W_ih, W_hh, b_ih, b_hh,
                fc1_w, fc1_b, fc2_w, fc2_b, t_steps=T):
    """Host-side layout prep; returns per-core input maps."""
    bf = ml_dtypes.bfloat16
    x = np.concatenate([np.asarray(m_text), np.asarray(m_audio),
                        np.asarray(m_video)], axis=-1).astype(np.float32)
    b_, t_, d_ = x.shape
    assert (b_, d_) == (B, D) and t_ == t_steps
    # x_T: [T, D+1(ones) padded to 4*128, B]
    xTf = np.zeros((t_steps, NKX * 128, B), np.float32)
    xTf[:, :D, :] = x.transpose(1, 2, 0)
    xTf[:, D, :] = 1.0
    xTf = np.ascontiguousarray(xTf.reshape(t_steps, NKX, 128, B)).astype(bf)

    W_ih = np.asarray(W_ih, np.float32)
    W_hh = np.asarray(W_hh, np.float32)
    bias = (np.asarray(b_ih) + np.asarray(b_hh)).astype(np.float32)
    fc1_w = np.asarray(fc1_w, np.float32)
    fc1_b = np.asarray(fc1_b, np.float32)
    fc2_w = np.asarray(fc2_w, np.float32)
    fc2_b = np.asarray(fc2_b, np.float32)

    in_maps = []
    gate_order = (0, 1, 2, 3)  # i, f, g, o (pytorch row-block order)
    for k in range(N_CORES):
        rows = np.concatenate(
            [np.arange(g * H + k * HSL, g * H + (k + 1) * HSL)
             for g in gate_order])
        wih_sl = W_ih[rows, :]            # [512, 409]
        whh_sl = W_hh[rows, :]            # [512, 1024]
        b_sl = bias[rows]                 # [512]
        wihT_k = np.zeros((NKX * 128, GSL), np.float32)
        wihT_k[:D, :] = wih_sl.T
        wihT_k[D, :] = b_sl
        wihT_k = np.ascontiguousarray(
            wihT_k.reshape(NKX, 128, GSL)).astype(bf)
        whhT_k = np.ascontiguousarray(
            whh_sl.T.reshape(NKH, 128, GSL)).astype(bf)

        cc = k % 4                        # C chunk (cores k and k+4 duplicate)
        crows = np.arange(cc * 128, (cc + 1) * 128)
        fc1wT_k = np.ascontiguousarray(
            fc1_w[crows, :].T.reshape(NKH, 128, 128)).astype(bf)
        fc1b_k = np.ascontiguousarray(fc1_b[crows].reshape(128, 1))
        fc2wT_k = np.ascontiguousarray(0.5 * fc2_w[:, crows].T).astype(bf)
        fc2b_k = np.ascontiguousarray(
            np.broadcast_to(fc2_b[None, :], (128, O))).astype(np.float32)

        in_maps.append({
            "xT": xTf,
            "wihT": wihT_k,
            "whhT": whhT_k,
            "fc1wT": fc1wT_k,
            "fc1b": fc1b_k,
            "fc2wT": fc2wT_k,
            "fc2b": fc2b_k,
        })
    return in_maps


def run(inputs, t_steps=T, trace=False):
    nc = _get_kernel(t_steps)
    in_maps = prep_inputs(
        inputs["m_text"], inputs["m_audio"], inputs["m_video"],
        inputs["W_ih"], inputs["W_hh"], inputs["b_ih"], inputs["b_hh"],
        inputs["fc1_w"], inputs["fc1_b"], inputs["fc2_w"], inputs["fc2_b"],
        t_steps=t_steps)
    res = run_bass_kernel_spmd(
        nc, in_maps, core_ids=list(range(N_CORES)), trace=trace)
    shards = [res.results[k]["out_sh"] for k in range(N_CORES)]
    full = np.concatenate(shards, axis=0)          # [(b t), O] token-major
    out = full.reshape(B, t_steps, O)
    return out, res


def kernel(**inputs) -> np.ndarray:
    t_steps = np.asarray(inputs["m_text"]).shape[1]
    out, _ = run(inputs, t_steps=t_steps)
    return out.astype(np.float32)



# revision 1
# speedup vs baseline: 1.1823x; 1.1823x over previous
"""EFLSTM Trainium2 kernel: 8-core tensor-parallel LSTM + fused head.

Strategy (8 NeuronCores, SPMD single program, per-core data differs):
  - Tensor-parallel over the 4H gate dimension: core k owns gate rows
    [k*128:(k+1)*128] of each of the four gates (i, f, g, o), so it computes
    h rows [k*128:(k+1)*128] each timestep.  A per-step AllGather rebuilds
    the full transposed hidden state h_T [H=1024, B=128] on every core.
  - Input projection x @ W_ih.T is folded into the recurrent PSUM
    accumulation (x is pre-transposed on the host with a ones-row so the
    bias is folded into the matmul as well).  The projection matmuls do not
    depend on h, so they execute during the AllGather latency window.
  - Matmul operands and the h exchange are bf16 (fp32 PSUM accumulation,
    fp32 cell state); gate nonlinearities are split per gate chunk so the
    c-update overlaps the o-gate matmuls.
  - FC head is fused per-step: fc1 (one 128-wide slice of C per core; cores
    k and k+4 duplicate a slice, fc2 weights pre-scaled by 0.5 so the final
    ReduceScatter sum is correct) + fc2 partials accumulate into a
    token-major buffer; after the loop one ReduceScatter + log_softmax on
    each core's token shard produces the output.
"""

import numpy as np
import ml_dtypes

import concourse.bacc as bacc
import concourse.mybir as mybir
import concourse.tile as tile
from concourse.bass_utils import run_bass_kernel_spmd

F32 = mybir.dt.float32
BF16 = mybir.dt.bfloat16
AF = mybir.ActivationFunctionType
ALU = mybir.AluOpType

N_CORES = 8
B = 128
T = 512
DIMS = (300, 74, 35)
D = sum(DIMS)  # 409
DP = D + 1     # 410: extra ones-row folds the gate bias into the matmul
H = 1024
G = 4 * H
C = 512
O = 7
HSL = H // N_CORES        # 128 h rows per core
GSL = 4 * HSL             # 512 gate rows per core
KC_X = [128, 128, 128, DP - 3 * 128]   # contraction chunks over D+1
NKX = len(KC_X)
NKH = H // 128            # 8 contraction chunks over H
TOK = B * T


NO_CC = False


def build_kernel(t_steps=T):
    nc = bacc.Bacc("TRN2", target_bir_lowering=False, debug=False,
                   num_devices=N_CORES)

    xT = nc.dram_tensor("xT", [t_steps, NKX, 128, B], BF16, kind="ExternalInput")
    wihT = nc.dram_tensor("wihT", [NKX, 128, GSL], BF16, kind="ExternalInput")
    whhT = nc.dram_tensor("whhT", [NKH, 128, GSL], BF16, kind="ExternalInput")
    fc1wT = nc.dram_tensor("fc1wT", [NKH, 128, 128], BF16, kind="ExternalInput")
    fc1b = nc.dram_tensor("fc1b", [128, 1], F32, kind="ExternalInput")
    fc2wT = nc.dram_tensor("fc2wT", [128, O], BF16, kind="ExternalInput")
    fc2b = nc.dram_tensor("fc2b", [128, O], F32, kind="ExternalInput")

    n_tok_loc = B * t_steps // N_CORES
    out_sh = nc.dram_tensor("out_sh", [n_tok_loc, O], F32, kind="ExternalOutput")

    with tile.TileContext(nc) as tc:
        with (
            tc.tile_pool(name="const", bufs=1) as const,
            tc.tile_pool(name="xtp", bufs=3) as xtp,
            tc.tile_pool(name="pg", bufs=2, space="PSUM") as pgp,
            tc.tile_pool(name="ph", bufs=2, space="PSUM") as php,
            tc.tile_pool(name="pf", bufs=2, space="PSUM") as pfp,
            tc.tile_pool(name="ew", bufs=2) as ewp,
            tc.tile_pool(name="htp", bufs=3) as htp,
            tc.tile_pool(name="dram", bufs=2, space="DRAM") as dramp,
            tc.tile_pool(name="dram1", bufs=1, space="DRAM") as dramp1,
        ):
            # ---- resident weights ----
            wih_sb = const.tile([128, NKX, GSL], BF16)
            nc.sync.dma_start(wih_sb[:], wihT[:].rearrange("k p g -> p k g"))
            whh_sb = const.tile([128, NKH, GSL], BF16)
            nc.sync.dma_start(whh_sb[:], whhT[:].rearrange("k p g -> p k g"))
            fc1_sb = const.tile([128, NKH, 128], BF16)
            nc.sync.dma_start(fc1_sb[:], fc1wT[:].rearrange("k p c -> p k c"))
            fc1b_sb = const.tile([128, 1], F32)
            nc.sync.dma_start(fc1b_sb[:], fc1b[:])
            fc2w_sb = const.tile([128, O], BF16)
            nc.sync.dma_start(fc2w_sb[:], fc2wT[:])
            fc2b_sb = const.tile([128, O], F32)
            nc.sync.dma_start(fc2b_sb[:], fc2b[:])

            c_sb = const.tile([128, 128], F32)   # persistent cell state slice

            out_part = dramp1.tile([n_tok_loc * N_CORES, O], F32)

            hT_prev = None
            gate_slices = [(mc * 128, (mc + 1) * 128) for mc in range(4)]

            for t in range(t_steps):
                # -- projection matmuls for step t (independent of h) --
                xts = xtp.tile([128, NKX, B], BF16)
                nc.sync.dma_start(xts[:], xT[t].rearrange("k p b -> p k b"))
                pg = pgp.tile([128, GSL], F32)
                for mc in range(4):
                    lo, hi = gate_slices[mc]
                    for kc in range(NKX):
                        kk = KC_X[kc]
                        nc.tensor.matmul(
                            pg[:, lo:hi],
                            wih_sb[:kk, kc, lo:hi],
                            xts[:kk, kc, :],
                            start=(kc == 0),
                            stop=(t == 0 and kc == NKX - 1),
                        )
                # -- recurrent matmuls (consume previous gathered h) --
                if t > 0:
                    for mc in range(4):
                        lo, hi = gate_slices[mc]
                        for kc in range(NKH):
                            nc.tensor.matmul(
                                pg[:, lo:hi],
                                whh_sb[:, kc, lo:hi],
                                hT_prev[:, kc, :],
                                start=False,
                                stop=(kc == NKH - 1),
                            )

                # -- head for step t-1 (emitted after rec so PE prioritizes rec) --
                if t > 0:
                    emit_head(nc, php, pfp, ewp, fc1_sb, fc1b_sb, fc2w_sb,
                              hT_prev, out_part, t - 1, t_steps)

                # -- elementwise: gates -> h slice (i,f,g,o gate chunk order;
                #    c-update runs while the o-gate matmuls finish) --
                gnl = ewp.tile([128, GSL], F32)
                nc.scalar.activation(gnl[:, 0:256], pg[:, 0:256], AF.Sigmoid)
                if t > 0:
                    fcs = ewp.tile([128, 128], F32)
                    nc.vector.tensor_mul(fcs[:], gnl[:, 128:256], c_sb[:])
                nc.scalar.activation(gnl[:, 256:384], pg[:, 256:384], AF.Tanh)
                ig = ewp.tile([128, 128], F32)
                nc.vector.tensor_mul(ig[:], gnl[:, 0:128], gnl[:, 256:384])
                if t == 0:
                    nc.vector.tensor_copy(c_sb[:], ig[:])
                else:
                    nc.vector.tensor_add(c_sb[:], fcs[:], ig[:])
                tc_t = ewp.tile([128, 128], F32)
                nc.scalar.activation(tc_t[:], c_sb[:], AF.Tanh)
                nc.scalar.activation(gnl[:, 384:512], pg[:, 384:512], AF.Sigmoid)
                h_sl = ewp.tile([128, 128], BF16)
                nc.vector.tensor_mul(h_sl[:], gnl[:, 384:512], tc_t[:])

                # -- exchange: AllGather h_T across the 8 cores --
                bnc_in = dramp.tile([128, B], BF16)
                nc.sync.dma_start(bnc_in[:], h_sl[:])
                bnc_out = dramp.tile(
                    [H, B], BF16, addr_space="Local" if NO_CC else "Shared")
                if NO_CC:
                    for kc in range(NKH):
                        nc.gpsimd.dma_start(
                            bnc_out[kc * 128:(kc + 1) * 128, :], bnc_in[:])
                else:
                    nc.gpsimd.collective_compute(
                        "AllGather",
                        ALU.bypass,
                        replica_groups=[list(range(N_CORES))],
                        ins=[bnc_in[:].opt()],
                        outs=[bnc_out[:].opt()],
                    )
                hT = htp.tile([128, NKH, B], BF16)
                for kc in range(NKH):
                    nc.sync.dma_start(hT[:, kc, :],
                                      bnc_out[kc * 128:(kc + 1) * 128, :])
                hT_prev = hT

            # head for the final step
            emit_head(nc, php, pfp, ewp, fc1_sb, fc1b_sb, fc2w_sb,
                      hT_prev, out_part, t_steps - 1, t_steps)

            # ---- tail: ReduceScatter fc2 partials, bias + log_softmax ----
            rs_out = dramp1.tile([n_tok_loc, O], F32)
            nc.gpsimd.collective_compute(
                "ReduceScatter",
                ALU.add,
                replica_groups=[list(range(N_CORES))],
                ins=[out_part[:].opt()],
                outs=[rs_out[:].opt()],
            )
            n_chunks = n_tok_loc // 128
            for ch in range(n_chunks):
                z = ewp.tile([128, O], F32)
                nc.sync.dma_start(z[:], rs_out[ch * 128:(ch + 1) * 128, :])
                zb = ewp.tile([128, O], F32)
                nc.vector.tensor_add(zb[:], z[:], fc2b_sb[:])
                mx = ewp.tile([128, 1], F32)
                nc.vector.reduce_max(mx[:], zb[:], axis=mybir.AxisListType.X)
                sh = ewp.tile([128, O], F32)
                nc.vector.tensor_scalar_sub(sh[:], zb[:], mx[:])
                ex = ewp.tile([128, O], F32)
                nc.scalar.activation(ex[:], sh[:], AF.Exp)
                sm = ewp.tile([128, 1], F32)
                nc.vector.reduce_sum(sm[:], ex[:], axis=mybir.AxisListType.X)
                lg = ewp.tile([128, 1], F32)
                nc.scalar.activation(lg[:], sm[:], AF.Ln)
                res = ewp.tile([128, O], F32)
                nc.vector.tensor_scalar_sub(res[:], sh[:], lg[:])
                nc.sync.dma_start(out_sh[ch * 128:(ch + 1) * 128, :], res[:])

    nc.compile()
    return nc


def emit_head(nc, php, pfp, ewp, fc1_sb, fc1b_sb, fc2w_sb, hT, out_part, t,
              t_steps):
    """fc1 (C slice) + relu + fc2 partial for timestep t, token-major store."""
    ph = php.tile([128, B], F32)
    for kc in range(NKH):
        nc.tensor.matmul(ph[:], fc1_sb[:, kc, :], hT[:, kc, :],
                         start=(kc == 0), stop=(kc == NKH - 1))
    hid = ewp.tile([128, B], BF16)
    nc.scalar.activation(hid[:], ph[:], AF.Relu, bias=fc1b_sb[:])
    pf = pfp.tile([128, O], F32)
    nc.tensor.matmul(pf[:], hid[:], fc2w_sb[:], start=True, stop=True)
    z = ewp.tile([128, O], F32)
    nc.vector.tensor_copy(z[:], pf[:])
    # out_part is token-major [(b t), O]
    nc.sync.dma_start(
        out_part[:].rearrange("(b t) o -> b t o", t=t_steps)[:, t, :], z[:])


_CACHED = {}


def _get_kernel(t_steps):
    if t_steps not in _CACHED:
        _CACHED[t_steps] = build_kernel(t_steps)
    return _CACHED[t_steps]


def prep_inputs(m_text, m_audio, m_video, W_ih, W_hh, b_ih, b_hh,
                fc1_w, fc1_b, fc2_w, fc2_b, t_steps=T):
    """Host-side layout prep; returns per-core input maps."""
    bf = ml_dtypes.bfloat16
    x = np.concatenate([np.asarray(m_text), np.asarray(m_audio),
                        np.asarray(m_video)], axis=-1).astype(np.float32)
    b_, t_, d_ = x.shape
    assert (b_, d_) == (B, D) and t_ == t_steps
    # x_T: [T, D+1(ones) padded to 4*128, B]
    xTf = np.zeros((t_steps, NKX * 128, B), np.float32)
    xTf[:, :D, :] = x.transpose(1, 2, 0)
    xTf[:, D, :] = 1.0
    xTf = np.ascontiguousarray(xTf.reshape(t_steps, NKX, 128, B)).astype(bf)

    W_ih = np.asarray(W_ih, np.float32)
    W_hh = np.asarray(W_hh, np.float32)
    bias = (np.asarray(b_ih) + np.asarray(b_hh)).astype(np.float32)
    fc1_w = np.asarray(fc1_w, np.float32)
    fc1_b = np.asarray(fc1_b, np.float32)
    fc2_w = np.asarray(fc2_w, np.float32)
    fc2_b = np.asarray(fc2_b, np.float32)

    in_maps = []
    gate_order = (0, 1, 2, 3)  # i, f, g, o (pytorch row-block order)
    for k in range(N_CORES):
        rows = np.concatenate(
            [np.arange(g * H + k * HSL, g * H + (k + 1) * HSL)
             for g in gate_order])
        wih_sl = W_ih[rows, :]            # [512, 409]
        whh_sl = W_hh[rows, :]            # [512, 1024]
        b_sl = bias[rows]                 # [512]
        wihT_k = np.zeros((NKX * 128, GSL), np.float32)
        wihT_k[:D, :] = wih_sl.T
        wihT_k[D, :] = b_sl
        wihT_k = np.ascontiguousarray(
            wihT_k.reshape(NKX, 128, GSL)).astype(bf)
        whhT_k = np.ascontiguousarray(
            whh_sl.T.reshape(NKH, 128, GSL)).astype(bf)

        cc = k % 4                        # C chunk (cores k and k+4 duplicate)
        crows = np.arange(cc * 128, (cc + 1) * 128)
        fc1wT_k = np.ascontiguousarray(
            fc1_w[crows, :].T.reshape(NKH, 128, 128)).astype(bf)
        fc1b_k = np.ascontiguousarray(fc1_b[crows].reshape(128, 1))
        fc2wT_k = np.ascontiguousarray(0.5 * fc2_w[:, crows].T).astype(bf)
        fc2b_k = np.ascontiguousarray(
            np.broadcast_to(fc2_b[None, :], (128, O))).astype(np.float32)

        in_maps.append({
            "xT": xTf,
            "wihT": wihT_k,
            "whhT": whhT_k,
            "fc1wT": fc1wT_k,
            "fc1b": fc1b_k,
            "fc2wT": fc2wT_k,
            "fc2b": fc2b_k,
        })
    return in_maps


def run(inputs, t_steps=T, trace=False):
    nc = _get_kernel(t_steps)
    in_maps = prep_inputs(
        inputs["m_text"], inputs["m_audio"], inputs["m_video"],
        inputs["W_ih"], inputs["W_hh"], inputs["b_ih"], inputs["b_hh"],
        inputs["fc1_w"], inputs["fc1_b"], inputs["fc2_w"], inputs["fc2_b"],
        t_steps=t_steps)
    res = run_bass_kernel_spmd(
        nc, in_maps, core_ids=list(range(N_CORES)), trace=trace)
    shards = [res.results[k]["out_sh"] for k in range(N_CORES)]
    full = np.concatenate(shards, axis=0)          # [(b t), O] token-major
    out = full.reshape(B, t_steps, O)
    return out, res


def kernel(**inputs) -> np.ndarray:
    t_steps = np.asarray(inputs["m_text"]).shape[1]
    out, _ = run(inputs, t_steps=t_steps)
    return out.astype(np.float32)



# revision 2
# speedup vs baseline: 57.7704x; 48.8607x over previous
"""EFLSTM Trainium2 kernel: 8-core tensor-parallel LSTM + fused head.

Strategy (8 NeuronCores, SPMD single program, per-core data differs):
  - Tensor-parallel over the 4H gate dimension: core k owns gate rows
    [k*128:(k+1)*128] of each of the four gates (i, f, g, o), so it computes
    h rows [k*128:(k+1)*128] each timestep.  A per-step AllGather rebuilds
    the full transposed hidden state h_T [H=1024, B=128] on every core.
  - Input projection x @ W_ih.T is folded into the recurrent PSUM
    accumulation (x is pre-transposed on the host with a ones-row so the
    bias is folded into the matmul as well).  The projection matmuls do not
    depend on h, so they execute during the AllGather latency window.
  - Matmul operands and the h exchange are bf16 (fp32 PSUM accumulation,
    fp32 cell state); gate nonlinearities are split per gate chunk so the
    c-update overlaps the o-gate matmuls.
  - FC head is fused per-step: fc1 (one 128-wide slice of C per core; cores
    k and k+4 duplicate a slice, fc2 weights pre-scaled by 0.5 so the final
    ReduceScatter sum is correct) + fc2 partials accumulate into a
    token-major buffer; after the loop one ReduceScatter + log_softmax on
    each core's token shard produces the output.
"""

import numpy as np
import ml_dtypes

import concourse.bacc as bacc
import concourse.mybir as mybir
import concourse.tile as tile
from concourse.bass_utils import run_bass_kernel_spmd

F32 = mybir.dt.float32
BF16 = mybir.dt.bfloat16
AF = mybir.ActivationFunctionType
ALU = mybir.AluOpType

N_CORES = 8
B = 128
T = 512
DIMS = (300, 74, 35)
D = sum(DIMS)  # 409
DP = D + 1     # 410: extra ones-row folds the gate bias into the matmul
H = 1024
G = 4 * H
C = 512
O = 7
HSL = H // N_CORES        # 128 h rows per core
GSL = 4 * HSL             # 512 gate rows per core
KC_X = [128, 128, 128, DP - 3 * 128]   # contraction chunks over D+1
NKX = len(KC_X)
NKH = H // 128            # 8 contraction chunks over H
TOK = B * T


import os
NO_CC = bool(int(os.environ.get("EF_NO_CC", "0")))


def build_kernel(t_steps=T):
    nc = bacc.Bacc("TRN2", target_bir_lowering=False, debug=False,
                   num_devices=N_CORES)

    xT = nc.dram_tensor("xT", [t_steps, NKX, 128, B], BF16, kind="ExternalInput")
    wihT = nc.dram_tensor("wihT", [NKX, 128, GSL], BF16, kind="ExternalInput")
    whhT = nc.dram_tensor("whhT", [NKH, 128, GSL], BF16, kind="ExternalInput")
    fc1wT = nc.dram_tensor("fc1wT", [NKH, 128, 128], BF16, kind="ExternalInput")
    fc1b = nc.dram_tensor("fc1b", [128, 1], F32, kind="ExternalInput")
    fc2wT = nc.dram_tensor("fc2wT", [128, O], BF16, kind="ExternalInput")
    fc2b = nc.dram_tensor("fc2b", [128, O], F32, kind="ExternalInput")

    n_tok_loc = B * t_steps // N_CORES
    out_sh = nc.dram_tensor("out_sh", [n_tok_loc, O], F32, kind="ExternalOutput")

    with tile.TileContext(nc) as tc:
        with (
            tc.tile_pool(name="const", bufs=1) as const,
            tc.tile_pool(name="xtp", bufs=3) as xtp,
            tc.tile_pool(name="pg", bufs=2, space="PSUM") as pgp,
            tc.tile_pool(name="ph", bufs=2, space="PSUM") as php,
            tc.tile_pool(name="pf", bufs=2, space="PSUM") as pfp,
            tc.tile_pool(name="ew", bufs=2) as ewp,
            tc.tile_pool(name="htp", bufs=3) as htp,
            tc.tile_pool(name="dram", bufs=2, space="DRAM") as dramp,
            tc.tile_pool(name="dram1", bufs=1, space="DRAM") as dramp1,
        ):
            # ---- resident weights ----
            wih_sb = const.tile([128, NKX, GSL], BF16)
            nc.sync.dma_start(wih_sb[:], wihT[:].rearrange("k p g -> p k g"))
            whh_sb = const.tile([128, NKH, GSL], BF16)
            nc.sync.dma_start(whh_sb[:], whhT[:].rearrange("k p g -> p k g"))
            fc1_sb = const.tile([128, NKH, 128], BF16)
            nc.sync.dma_start(fc1_sb[:], fc1wT[:].rearrange("k p c -> p k c"))
            fc1b_sb = const.tile([128, 1], F32)
            nc.sync.dma_start(fc1b_sb[:], fc1b[:])
            fc2w_sb = const.tile([128, O], BF16)
            nc.sync.dma_start(fc2w_sb[:], fc2wT[:])
            fc2b_sb = const.tile([128, O], F32)
            nc.sync.dma_start(fc2b_sb[:], fc2b[:])

            c_sb = const.tile([128, 128], F32)   # persistent cell state slice

            out_part = dramp1.tile([n_tok_loc * N_CORES, O], F32)

            hT_prev = None
            gate_slices = [(mc * 128, (mc + 1) * 128) for mc in range(4)]

            for t in range(t_steps):
                # -- projection matmuls for step t (independent of h) --
                xts = xtp.tile([128, NKX, B], BF16)
                nc.sync.dma_start(xts[:], xT[t].rearrange("k p b -> p k b"))
                pg = pgp.tile([128, GSL], F32)
                for mc in range(4):
                    lo, hi = gate_slices[mc]
                    for kc in range(NKX):
                        kk = KC_X[kc]
                        nc.tensor.matmul(
                            pg[:, lo:hi],
                            wih_sb[:kk, kc, lo:hi],
                            xts[:kk, kc, :],
                            start=(kc == 0),
                            stop=(t == 0 and kc == NKX - 1),
                        )
                # -- recurrent matmuls (consume previous gathered h) --
                if t > 0:
                    for mc in range(4):
                        lo, hi = gate_slices[mc]
                        for kc in range(NKH):
                            nc.tensor.matmul(
                                pg[:, lo:hi],
                                whh_sb[:, kc, lo:hi],
                                hT_prev[:, kc, :],
                                start=False,
                                stop=(kc == NKH - 1),
                            )

                # -- head for step t-1 (emitted after rec so PE prioritizes rec) --
                if t > 0:
                    emit_head(nc, php, pfp, ewp, fc1_sb, fc1b_sb, fc2w_sb,
                              hT_prev, out_part, t - 1, t_steps)

                # -- elementwise: gates -> h slice (i,f,g,o gate chunk order;
                #    c-update runs while the o-gate matmuls finish) --
                gnl = ewp.tile([128, GSL], F32)
                nc.scalar.activation(gnl[:, 0:256], pg[:, 0:256], AF.Sigmoid)
                if t > 0:
                    fcs = ewp.tile([128, 128], F32)
                    nc.vector.tensor_mul(fcs[:], gnl[:, 128:256], c_sb[:])
                nc.scalar.activation(gnl[:, 256:384], pg[:, 256:384], AF.Tanh)
                ig = ewp.tile([128, 128], F32)
                nc.vector.tensor_mul(ig[:], gnl[:, 0:128], gnl[:, 256:384])
                if t == 0:
                    nc.vector.tensor_copy(c_sb[:], ig[:])
                else:
                    nc.vector.tensor_add(c_sb[:], fcs[:], ig[:])
                tc_t = ewp.tile([128, 128], F32)
                nc.scalar.activation(tc_t[:], c_sb[:], AF.Tanh)
                nc.scalar.activation(gnl[:, 384:512], pg[:, 384:512], AF.Sigmoid)
                h_sl = ewp.tile([128, 128], BF16)
                nc.vector.tensor_mul(h_sl[:], gnl[:, 384:512], tc_t[:])

                # -- exchange: AllGather h_T across the 8 cores --
                bnc_in = dramp.tile([128, B], BF16)
                nc.sync.dma_start(bnc_in[:], h_sl[:])
                bnc_out = dramp.tile(
                    [H, B], BF16, addr_space="Local" if NO_CC else "Shared")
                if NO_CC:
                    for kc in range(NKH):
                        nc.gpsimd.dma_start(
                            bnc_out[kc * 128:(kc + 1) * 128, :], bnc_in[:])
                else:
                    nc.gpsimd.collective_compute(
                        "AllGather",
                        ALU.bypass,
                        replica_groups=[list(range(N_CORES))],
                        ins=[bnc_in[:].opt()],
                        outs=[bnc_out[:].opt()],
                    )
                hT = htp.tile([128, NKH, B], BF16)
                for kc in range(NKH):
                    nc.sync.dma_start(hT[:, kc, :],
                                      bnc_out[kc * 128:(kc + 1) * 128, :])
                hT_prev = hT

            # head for the final step
            emit_head(nc, php, pfp, ewp, fc1_sb, fc1b_sb, fc2w_sb,
                      hT_prev, out_part, t_steps - 1, t_steps)

            # ---- tail: ReduceScatter fc2 partials, bias + log_softmax ----
            rs_out = dramp1.tile([n_tok_loc, O], F32)
            nc.gpsimd.collective_compute(
                "ReduceScatter",
                ALU.add,
                replica_groups=[list(range(N_CORES))],
                ins=[out_part[:].opt()],
                outs=[rs_out[:].opt()],
            )
            n_chunks = n_tok_loc // 128
            for ch in range(n_chunks):
                z = ewp.tile([128, O], F32)
                nc.sync.dma_start(z[:], rs_out[ch * 128:(ch + 1) * 128, :])
                zb = ewp.tile([128, O], F32)
                nc.vector.tensor_add(zb[:], z[:], fc2b_sb[:])
                mx = ewp.tile([128, 1], F32)
                nc.vector.reduce_max(mx[:], zb[:], axis=mybir.AxisListType.X)
                sh = ewp.tile([128, O], F32)
                nc.vector.tensor_scalar_sub(sh[:], zb[:], mx[:])
                ex = ewp.tile([128, O], F32)
                nc.scalar.activation(ex[:], sh[:], AF.Exp)
                sm = ewp.tile([128, 1], F32)
                nc.vector.reduce_sum(sm[:], ex[:], axis=mybir.AxisListType.X)
                lg = ewp.tile([128, 1], F32)
                nc.scalar.activation(lg[:], sm[:], AF.Ln)
                res = ewp.tile([128, O], F32)
                nc.vector.tensor_scalar_sub(res[:], sh[:], lg[:])
                nc.sync.dma_start(out_sh[ch * 128:(ch + 1) * 128, :], res[:])

    nc.compile()
    return nc


def emit_head(nc, php, pfp, ewp, fc1_sb, fc1b_sb, fc2w_sb, hT, out_part, t,
              t_steps):
    """fc1 (C slice) + relu + fc2 partial for timestep t, token-major store."""
    ph = php.tile([128, B], F32)
    for kc in range(NKH):
        nc.tensor.matmul(ph[:], fc1_sb[:, kc, :], hT[:, kc, :],
                         start=(kc == 0), stop=(kc == NKH - 1))
    hid = ewp.tile([128, B], BF16)
    nc.scalar.activation(hid[:], ph[:], AF.Relu, bias=fc1b_sb[:])
    pf = pfp.tile([128, O], F32)
    nc.tensor.matmul(pf[:], hid[:], fc2w_sb[:], start=True, stop=True)
    z = ewp.tile([128, O], F32)
    nc.vector.tensor_copy(z[:], pf[:])
    # out_part is token-major [(b t), O]
    nc.sync.dma_start(
        out_part[:].rearrange("(b t) o -> b t o", t=t_steps)[:, t, :], z[:])


_CACHED = {}


def _get_kernel(t_steps):
    if t_steps not in _CACHED:
        _CACHED[t_steps] = build_kernel(t_steps)
    return _CACHED[t_steps]


def prep_inputs(m_text, m_audio, m_video, W_ih, W_hh, b_ih, b_hh,
                fc1_w, fc1_b, fc2_w, fc2_b, t_steps=T):
    """Host-side layout prep; returns per-core input maps."""
    bf = ml_dtypes.bfloat16
    x = np.concatenate([np.asarray(m_text), np.asarray(m_audio),
                        np.asarray(m_video)], axis=-1).astype(np.float32)
    b_, t_, d_ = x.shape
    assert (b_, d_) == (B, D) and t_ == t_steps
    # x_T: [T, D+1(ones) padded to 4*128, B]
    xTf = np.zeros((t_steps, NKX * 128, B), np.float32)
    xTf[:, :D, :] = x.transpose(1, 2, 0)
    xTf[:, D, :] = 1.0
    xTf = np.ascontiguousarray(xTf.reshape(t_steps, NKX, 128, B)).astype(bf)

    W_ih = np.asarray(W_ih, np.float32)
    W_hh = np.asarray(W_hh, np.float32)
    bias = (np.asarray(b_ih) + np.asarray(b_hh)).astype(np.float32)
    fc1_w = np.asarray(fc1_w, np.float32)
    fc1_b = np.asarray(fc1_b, np.float32)
    fc2_w = np.asarray(fc2_w, np.float32)
    fc2_b = np.asarray(fc2_b, np.float32)

    in_maps = []
    gate_order = (0, 1, 2, 3)  # i, f, g, o (pytorch row-block order)
    for k in range(N_CORES):
        rows = np.concatenate(
            [np.arange(g * H + k * HSL, g * H + (k + 1) * HSL)
             for g in gate_order])
        wih_sl = W_ih[rows, :]            # [512, 409]
        whh_sl = W_hh[rows, :]            # [512, 1024]
        b_sl = bias[rows]                 # [512]
        wihT_k = np.zeros((NKX * 128, GSL), np.float32)
        wihT_k[:D, :] = wih_sl.T
        wihT_k[D, :] = b_sl
        wihT_k = np.ascontiguousarray(
            wihT_k.reshape(NKX, 128, GSL)).astype(bf)
        whhT_k = np.ascontiguousarray(
            whh_sl.T.reshape(NKH, 128, GSL)).astype(bf)

        cc = k % 4                        # C chunk (cores k and k+4 duplicate)
        crows = np.arange(cc * 128, (cc + 1) * 128)
        fc1wT_k = np.ascontiguousarray(
            fc1_w[crows, :].T.reshape(NKH, 128, 128)).astype(bf)
        fc1b_k = np.ascontiguousarray(fc1_b[crows].reshape(128, 1))
        fc2wT_k = np.ascontiguousarray(0.5 * fc2_w[:, crows].T).astype(bf)
        fc2b_k = np.ascontiguousarray(
            np.broadcast_to(fc2_b[None, :], (128, O))).astype(np.float32)

        in_maps.append({
            "xT": xTf,
            "wihT": wihT_k,
            "whhT": whhT_k,
            "fc1wT": fc1wT_k,
            "fc1b": fc1b_k,
            "fc2wT": fc2wT_k,
            "fc2b": fc2b_k,
        })
    return in_maps


def run(inputs, t_steps=T, trace=False):
    nc = _get_kernel(t_steps)
    in_maps = prep_inputs(
        inputs["m_text"], inputs["m_audio"], inputs["m_video"],
        inputs["W_ih"], inputs["W_hh"], inputs["b_ih"], inputs["b_hh"],
        inputs["fc1_w"], inputs["fc1_b"], inputs["fc2_w"], inputs["fc2_b"],
        t_steps=t_steps)
    res = run_bass_kernel_spmd(
        nc, in_maps, core_ids=list(range(N_CORES)), trace=trace)
    shards = [res.results[k]["out_sh"] for k in range(N_CORES)]
    full = np.concatenate(shards, axis=0)          # [(b t), O] token-major
    out = full.reshape(B, t_steps, O)
    return out, res


def kernel(**inputs) -> np.ndarray:
    t_steps = np.asarray(inputs["m_text"]).shape[1]
    out, _ = run(inputs, t_steps=t_steps)
    return out.astype(np.float32)

